# revision 4
# baseline (speedup 1.0000x reference)
"""Transformer encoder layer (LN -> MHA -> residual -> LN -> MLP -> residual)
on 8 Trainium2 NeuronCores.

Sharding: token-parallel over the 4096 (batch*seq) tokens, 512 query-tokens
per core; the 4 cores sharing a batch each redundantly compute the full
2048-token K/V for that batch, so no collectives are needed.

On-chip layout: activations are kept feature-major ("transposed", [d, token])
so every matmul contracts along the partition dim with weights in natural
[d_in, d_out] layout.  Softmax is computed unnormalized (scores are bounded,
so plain exp is numerically safe and algebraically identical); the denominator
comes for free from a ones-column appended to V, and the division is applied
in place to the tiny per-head attention accumulator.

LayerNorm gains/biases are folded into the following projections on the host
(exact algebra: (g*xhat+b) @ W = xhat @ (diag(g) W) + b @ W).
"""

import numpy as np

import concourse.bass as bass
import concourse.mybir as mybir
from concourse import bacc
from concourse.tile import TileContext
from concourse.bass_utils import run_bass_kernel_spmd
from concourse.masks import make_identity

F32 = mybir.dt.float32
AF = mybir.ActivationFunctionType
ALU = mybir.AluOpType

B, S, D = 2, 2048, 1024
H, HD = 16, 64
DFF = 4 * D
NCORES = 8
QT = 512           # query tokens per core
NCHUNK = S // 512  # kv chunks of 512 tokens
EPS = 1e-5


def _ln_transpose(nc, lnp, psT, ident, eps, x_src, hT, from_sbuf=False):
    """LayerNorm 512 tokens and write the transposed [d, token] result into
    hT ([128, 8, 512]).  x_src: DRAM AP rows [512, D] or SBUF tile view
    [128, 4, D]."""
    for st in range(4):
        if from_sbuf:
            xt = x_src[:, st, :]
        else:
            xt = lnp.tile([128, D], F32, tag="ln_x")
            nc.sync.dma_start(out=xt, in_=x_src[st * 128:(st + 1) * 128, :])
        stats = lnp.tile([128, 2, 6], F32, tag="ln_st")
        nc.vector.bn_stats(stats[:, 0, :], xt[:, 0:512])
        nc.vector.bn_stats(stats[:, 1, :], xt[:, 512:1024])
        mv = lnp.tile([128, 2], F32, tag="ln_mv")
        nc.vector.bn_aggr(mv, stats)
        sd = lnp.tile([128, 1], F32, tag="ln_sd")
        nc.scalar.activation(sd, mv[:, 1:2], AF.Sqrt, bias=eps[:, 0:1])
        rstd = lnp.tile([128, 1], F32, tag="ln_rs")
        nc.vector.reciprocal(rstd, sd)
        h = lnp.tile([128, D], F32, tag="ln_h")
        nc.vector.tensor_scalar(h, xt, mv[:, 0:1], rstd[:, 0:1], ALU.subtract, ALU.mult)
        for dt in range(8):
            pst = psT.tile([128, 128], F32, tag="tp")
            nc.tensor.transpose(pst, h[:, dt * 128:(dt + 1) * 128], ident)
            nc.vector.tensor_copy(hT[:, dt, st * 128:(st + 1) * 128], pst)


def _build():
    nc = bacc.Bacc(None, target_bir_lowering=False)

    XB = nc.declare_dram_parameter("xb", [S, D], F32, isOutput=False)
    XQ = nc.declare_dram_parameter("xq", [QT, D], F32, isOutput=False)
    WQ = nc.declare_dram_parameter("wq", [D, D], F32, isOutput=False)
    WK = nc.declare_dram_parameter("wk", [D, D], F32, isOutput=False)
    WV = nc.declare_dram_parameter("wv", [D, D], F32, isOutput=False)
    WO = nc.declare_dram_parameter("wo", [D, D], F32, isOutput=False)
    W1 = nc.declare_dram_parameter("w1", [D, DFF], F32, isOutput=False)
    W2 = nc.declare_dram_parameter("w2", [DFF, D], F32, isOutput=False)
    BQ = nc.declare_dram_parameter("bq", [D], F32, isOutput=False)
    BK = nc.declare_dram_parameter("bk", [D], F32, isOutput=False)
    BV = nc.declare_dram_parameter("bv", [D], F32, isOutput=False)
    BO = nc.declare_dram_parameter("bo", [D], F32, isOutput=False)
    B1 = nc.declare_dram_parameter("b1", [DFF], F32, isOutput=False)
    B2 = nc.declare_dram_parameter("b2", [D], F32, isOutput=False)
    Y = nc.declare_dram_parameter("y", [QT, D], F32, isOutput=True)

    with TileContext(nc) as tc:
        with (
            tc.tile_pool(name="const", bufs=1) as cpool,
            tc.tile_pool(name="accp", bufs=1) as accp,
            tc.tile_pool(name="x2p", bufs=1) as x2p,
        ):
            ident = cpool.tile([128, 128], F32)
            make_identity(nc, ident)
            eps = cpool.tile([128, 1], F32)
            nc.vector.memset(eps, EPS)
            ones64 = cpool.tile([1, 64], F32)
            nc.vector.memset(ones64, 1.0)
            bqT = cpool.tile([128, 8], F32)
            nc.sync.dma_start(out=bqT, in_=BQ[:].rearrange("(t p) -> p t", p=128))
            bkT = cpool.tile([128, 8], F32)
            nc.sync.dma_start(out=bkT, in_=BK[:].rearrange("(t p) -> p t", p=128))
            b1T = cpool.tile([128, 32], F32)
            nc.sync.dma_start(out=b1T, in_=B1[:].rearrange("(t p) -> p t", p=128))
            bv_bc = cpool.tile([128, D], F32)
            nc.sync.dma_start(out=bv_bc, in_=BV[:].partition_broadcast(128))
            bo_bc = cpool.tile([128, D], F32)
            nc.sync.dma_start(out=bo_bc, in_=BO[:].partition_broadcast(128))
            b2_bc = cpool.tile([128, D], F32)
            nc.sync.dma_start(out=b2_bc, in_=B2[:].partition_broadcast(128))

            acc = accp.tile([65, 16, 512], F32)  # unnormalized attn^T + denom row
            x2 = x2p.tile([128, 4, D], F32)      # post-attention residual stream

            # ---- projections + attention, streamed over kv chunks ----
            with (
                tc.tile_pool(name="qp", bufs=1) as qp,
                tc.tile_pool(name="lnp", bufs=2) as lnp,
                tc.tile_pool(name="hTp", bufs=2) as hTp,
                tc.tile_pool(name="ktp", bufs=1) as ktp,
                tc.tile_pool(name="vp", bufs=2) as vp,
                tc.tile_pool(name="wsm", bufs=4) as wsm,
                tc.tile_pool(name="wvp", bufs=1) as wvp,
                tc.tile_pool(name="pp", bufs=4) as ppl,
                tc.tile_pool(name="psK", bufs=2, space="PSUM") as psK,
                tc.tile_pool(name="psV", bufs=1, space="PSUM") as psV,
                tc.tile_pool(name="psS", bufs=2, space="PSUM") as psS,
                tc.tile_pool(name="psA", bufs=1, space="PSUM") as psA,
                tc.tile_pool(name="psT", bufs=2, space="PSUM") as psT,
            ):
                Q_sb = qp.tile([128, 8, 512], F32)  # Q^T [hd, q]

                # Q projection from the core's own tokens
                hqT = hTp.tile([128, 8, 512], F32, tag="hT")
                _ln_transpose(nc, lnp, psT, ident, eps, XQ[:], hqT)
                for ht in range(8):
                    psq = psK.tile([128, 512], F32, tag="psK")
                    for dt in range(8):
                        wt = wsm.tile([128, 128], F32, tag="w")
                        nc.sync.dma_start(
                            out=wt,
                            in_=WQ[dt * 128:(dt + 1) * 128, ht * 128:(ht + 1) * 128],
                        )
                        nc.tensor.matmul(
                            psq, wt, hqT[:, dt, :], start=(dt == 0), stop=(dt == 7)
                        )
                    nc.vector.tensor_scalar_add(Q_sb[:, ht, :], psq, bqT[:, ht:ht + 1])

                for kc in range(NCHUNK):
                    hT = hTp.tile([128, 8, 512], F32, tag="hT")
                    _ln_transpose(nc, lnp, psT, ident, eps, XB[kc * 512:(kc + 1) * 512, :], hT)

                    # K^T chunk [hd, 512]
                    KT = ktp.tile([128, 8, 512], F32, tag="KT")
                    for ht in range(8):
                        psk = psK.tile([128, 512], F32, tag="psK")
                        for dt in range(8):
                            wt = wsm.tile([128, 128], F32, tag="w")
                            nc.sync.dma_start(
                                out=wt,
                                in_=WK[dt * 128:(dt + 1) * 128, ht * 128:(ht + 1) * 128],
                            )
                            nc.tensor.matmul(
                                psk, wt, hT[:, dt, :], start=(dt == 0), stop=(dt == 7)
                            )
                        nc.vector.tensor_scalar_add(KT[:, ht, :], psk, bkT[:, ht:ht + 1])

                    # V chunk, natural layout [token, head, hd] + ones column
                    V = vp.tile([128, 4, 16, 65], F32, tag="V")
                    nc.vector.memset(V[:, :, :, 64:65], 1.0)
                    for hc in range(2):
                        wv_sb = wvp.tile([128, 8, 512], F32, tag="wv")
                        nc.sync.dma_start(
                            out=wv_sb,
                            in_=WV[:, hc * 512:(hc + 1) * 512].rearrange(
                                "(t p) n -> p t n", p=128
                            ),
                        )
                        for st in range(4):
                            psv = psV.tile([128, 512], F32, tag="psV")
                            for dt in range(8):
                                nc.tensor.matmul(
                                    psv,
                                    hT[:, dt, st * 128:(st + 1) * 128],
                                    wv_sb[:, dt, :],
                                    start=(dt == 0),
                                    stop=(dt == 7),
                                )
                            nc.vector.tensor_add(
                                V[:, st, hc * 8:(hc + 1) * 8, 0:64],
                                psv.rearrange("p (h d) -> p h d", h=8),
                                bv_bc[:, hc * 512:(hc + 1) * 512].rearrange(
                                    "p (h d) -> p h d", h=8
                                ),
                            )

                    # attention for this kv chunk
                    for h in range(H):
                        ko = (h % 2) * 64
                        kj = h // 2
                        p_tiles = []
                        for kt in range(4):
                            pss = psS.tile([128, 512], F32, tag="psS")
                            nc.tensor.matmul(
                                pss,
                                KT[ko:ko + 64, kj, kt * 128:(kt + 1) * 128],
                                Q_sb[ko:ko + 64, kj, :],
                                start=True,
                                stop=True,
                            )
                            P = ppl.tile([128, 512], F32, tag="P")
                            nc.scalar.activation(P, pss, AF.Exp, scale=0.125)
                            p_tiles.append(P)
                        psa = psA.tile([65, 512], F32, tag="psA")
                        for kt in range(4):
                            nc.tensor.matmul(
                                psa, V[:, kt, h, :], p_tiles[kt],
                                start=(kt == 0), stop=(kt == 3),
                            )
                        if kc == 0:
                            nc.vector.tensor_copy(acc[:, h, :], psa)
                        else:
                            nc.vector.tensor_add(acc[:, h, :], acc[:, h, :], psa)

            # ---- softmax normalization (in place) ----
            with (
                tc.tile_pool(name="dsm", bufs=4) as dsm,
                tc.tile_pool(name="psRB", bufs=2, space="PSUM") as psRB,
            ):
                for h in range(H):
                    r = dsm.tile([1, 512], F32, tag="r")
                    nc.vector.reciprocal(r, acc[64:65, h, :])
                    rb_ps = psRB.tile([64, 512], F32, tag="rb")
                    nc.tensor.matmul(rb_ps, ones64, r, start=True, stop=True)
                    rb = dsm.tile([64, 512], F32, tag="rb_sb")
                    nc.scalar.copy(rb, rb_ps)
                    nc.vector.tensor_mul(acc[0:64, h, :], acc[0:64, h, :], rb)

            # ---- out-projection + residual ----
            with (
                tc.tile_pool(name="xqp", bufs=1) as xqp,
                tc.tile_pool(name="dwo", bufs=3) as dwo,
                tc.tile_pool(name="dtmp", bufs=4) as dtmp,
                tc.tile_pool(name="psO", bufs=4, space="PSUM") as psO,
            ):
                xq_sb = xqp.tile([128, 4, D], F32)
                nc.sync.dma_start(
                    out=xq_sb, in_=XQ[:].rearrange("(t p) n -> p t n", p=128)
                )
                for c in range(2):
                    po = [psO.tile([128, 512], F32, tag="psO", name=f"po{c}_{i}") for i in range(4)]
                    for h in range(H):
                        wot = dwo.tile([64, 512], F32, tag="wo")
                        nc.sync.dma_start(
                            out=wot,
                            in_=WO[h * 64:(h + 1) * 64, c * 512:(c + 1) * 512],
                        )
                        for qt in range(4):
                            nc.tensor.matmul(
                                po[qt], acc[0:64, h, qt * 128:(qt + 1) * 128], wot,
                                start=(h == 0), stop=(h == H - 1),
                            )
                    for qt in range(4):
                        t1 = dtmp.tile([128, 512], F32, tag="t1")
                        nc.vector.tensor_add(
                            t1, po[qt], bo_bc[:, c * 512:(c + 1) * 512]
                        )
                        nc.vector.tensor_add(
                            x2[:, qt, c * 512:(c + 1) * 512],
                            t1,
                            xq_sb[:, qt, c * 512:(c + 1) * 512],
                        )

            # ---- LN2 + MLP + residual ----
            with (
                tc.tile_pool(name="lnp2", bufs=2) as lnp2,
                tc.tile_pool(name="h2p", bufs=1) as h2p,
                tc.tile_pool(name="gp", bufs=1) as gp,
                tc.tile_pool(name="wfp", bufs=3) as wfp,
                tc.tile_pool(name="yp", bufs=2) as yp,
                tc.tile_pool(name="psT2", bufs=2, space="PSUM") as psT2,
                tc.tile_pool(name="psF", bufs=2, space="PSUM") as psF,
                tc.tile_pool(name="psY", bufs=4, space="PSUM") as psY,
            ):
                h2T = h2p.tile([128, 8, 512], F32)
                _ln_transpose(nc, lnp2, psT2, ident, eps, x2, h2T, from_sbuf=True)

                # MLP1: gelu(h2 @ w1 + b1), transposed output [dff, q]
                G = gp.tile([128, 32, 512], F32)
                for ft in range(32):
                    psf = psF.tile([128, 512], F32, tag="psF")
                    for dt in range(8):
                        w1t = wfp.tile([128, 128], F32, tag="w1")
                        nc.sync.dma_start(
                            out=w1t,
                            in_=W1[dt * 128:(dt + 1) * 128, ft * 128:(ft + 1) * 128],
                        )
                        nc.tensor.matmul(
                            psf, w1t, h2T[:, dt, :], start=(dt == 0), stop=(dt == 7)
                        )
                    nc.scalar.activation(
                        G[:, ft, :], psf, AF.Gelu, bias=b1T[:, ft:ft + 1]
                    )

                # MLP2: y = G^T @ w2 + b2 + x2
                for c in range(2):
                    py = [psY.tile([128, 512], F32, tag="psY", name=f"py{c}_{i}") for i in range(4)]
                    for ft in range(32):
                        w2t = wfp.tile([128, 512], F32, tag="w2")
                        nc.sync.dma_start(
                            out=w2t,
                            in_=W2[ft * 128:(ft + 1) * 128, c * 512:(c + 1) * 512],
                        )
                        for qt in range(4):
                            nc.tensor.matmul(
                                py[qt], G[:, ft, qt * 128:(qt + 1) * 128], w2t,
                                start=(ft == 0), stop=(ft == 31),
                            )
                    for qt in range(4):
                        t1 = yp.tile([128, 512], F32, tag="yt1")
                        nc.vector.tensor_add(
                            t1, py[qt], b2_bc[:, c * 512:(c + 1) * 512]
                        )
                        yt = yp.tile([128, 512], F32, tag="yt2")
                        nc.vector.tensor_add(
                            yt, t1, x2[:, qt, c * 512:(c + 1) * 512]
                        )
                        nc.sync.dma_start(
                            out=Y[qt * 128:(qt + 1) * 128, c * 512:(c + 1) * 512],
                            in_=yt,
                        )

    nc.compile()
    return nc


_NC = None


def _get_nc():
    global _NC
    if _NC is None:
        _NC = _build()
    return _NC


def kernel(x, ln1_g, ln1_b, wq, bq, wk, bk, wv, bv, wo, bo, w1, b1, w2, b2, ln2_g, ln2_b):
    f32 = lambda a: np.ascontiguousarray(np.asarray(a, dtype=np.float32))
    x = f32(x)
    ln1_g, ln1_b = f32(ln1_g), f32(ln1_b)
    ln2_g, ln2_b = f32(ln2_g), f32(ln2_b)
    wq, wk, wv, wo = f32(wq), f32(wk), f32(wv), f32(wo)
    w1, w2 = f32(w1), f32(w2)
    bq, bk, bv, bo, b1, b2 = f32(bq), f32(bk), f32(bv), f32(bo), f32(b1), f32(b2)

    # Fold LayerNorm affine params into the following projections (exact).
    wq_e = f32(ln1_g[:, None] * wq)
    wk_e = f32(ln1_g[:, None] * wk)
    wv_e = f32(ln1_g[:, None] * wv)
    bq_e = f32(bq + ln1_b @ wq)
    bk_e = f32(bk + ln1_b @ wk)
    bv_e = f32(bv + ln1_b @ wv)
    w1_e = f32(ln2_g[:, None] * w1)
    b1_e = f32(b1 + ln2_b @ w1)

    common = {
        "wq": wq_e, "wk": wk_e, "wv": wv_e, "wo": wo,
        "w1": w1_e, "w2": w2,
        "bq": bq_e, "bk": bk_e, "bv": bv_e, "bo": bo,
        "b1": b1_e, "b2": b2,
    }
    in_maps = []
    for c in range(NCORES):
        b = c // 4
        qoff = (c % 4) * QT
        m = dict(common)
        m["xb"] = np.ascontiguousarray(x[b])
        m["xq"] = np.ascontiguousarray(x[b, qoff:qoff + QT])
        in_maps.append(m)

    nc = _get_nc()
    res = run_bass_kernel_spmd(nc, in_maps, core_ids=list(range(NCORES)))

    y = np.empty((B, S, D), dtype=np.float32)
    for c in range(NCORES):
        b = c // 4
        qoff = (c % 4) * QT
        y[b, qoff:qoff + QT] = res.results[c]["y"]
    return y


# revision 7
# speedup vs baseline: 1.8432x; 1.8432x over previous
"""Transformer encoder layer (LN -> MHA -> residual -> LN -> MLP -> residual)
on 8 Trainium2 NeuronCores.

Sharding: token-parallel over the 4096 (batch*seq) tokens, 512 query-tokens
per core; the 4 cores sharing a batch each redundantly compute the full
2048-token K/V for that batch, so no collectives are needed.

On-chip layout: activations are kept feature-major ("transposed", [d, token])
so every matmul contracts along the partition dim with weights in natural
[d_in, d_out] layout.  Softmax is computed unnormalized (scores are bounded,
so plain exp is numerically safe and algebraically identical); the denominator
comes for free from a ones-column appended to V, and the division is applied
in place to the tiny per-head attention accumulator.

LayerNorm gains/biases are folded into the following projections on the host
(exact algebra: (g*xhat+b) @ W = xhat @ (diag(g) W) + b @ W).
"""

import numpy as np

import concourse.bass as bass
import concourse.mybir as mybir
from concourse import bacc
from concourse.tile import TileContext
from concourse.bass_utils import run_bass_kernel_spmd
from concourse.masks import make_identity

F32 = mybir.dt.float32
F32R = mybir.dt.float32r
MMDT = F32R  # dtype for matmul operands (float32r = full-rate PE)
AF = mybir.ActivationFunctionType
ALU = mybir.AluOpType

B, S, D = 2, 2048, 1024
H, HD = 16, 64
DFF = 4 * D
NCORES = 8
QT = 512           # query tokens per core
NCHUNK = S // 512  # kv chunks of 512 tokens
EPS = 1e-5


def _ln_transpose(nc, lnp, psT, ident, eps, x_src, hT, from_sbuf=False):
    """LayerNorm 512 tokens and write the transposed [d, token] result into
    hT ([128, 8, 512]).  x_src: DRAM AP rows [512, D] or SBUF tile view
    [128, 4, D]."""
    for st in range(4):
        if from_sbuf:
            xt = x_src[:, st, :]
        else:
            xt = lnp.tile([128, D], F32, tag="ln_x")
            nc.sync.dma_start(out=xt, in_=x_src[st * 128:(st + 1) * 128, :])
        stats = lnp.tile([128, 2, 6], F32, tag="ln_st")
        nc.vector.bn_stats(stats[:, 0, :], xt[:, 0:512])
        nc.vector.bn_stats(stats[:, 1, :], xt[:, 512:1024])
        mv = lnp.tile([128, 2], F32, tag="ln_mv")
        nc.vector.bn_aggr(mv, stats)
        sd = lnp.tile([128, 1], F32, tag="ln_sd")
        nc.scalar.activation(sd, mv[:, 1:2], AF.Sqrt, bias=eps[:, 0:1])
        rstd = lnp.tile([128, 1], F32, tag="ln_rs")
        nc.vector.reciprocal(rstd, sd)
        h = lnp.tile([128, D], F32, tag="ln_h")
        nc.vector.tensor_scalar(h, xt, mv[:, 0:1], rstd[:, 0:1], ALU.subtract, ALU.mult)
        for dt in range(8):
            pst = psT.tile([128, 128], F32, tag="tp")
            nc.tensor.transpose(pst, h[:, dt * 128:(dt + 1) * 128], ident)
            nc.vector.tensor_copy(hT[:, dt, st * 128:(st + 1) * 128], pst)


def _build():
    nc = bacc.Bacc(None, target_bir_lowering=False)

    XB = nc.declare_dram_parameter("xb", [S, D], F32, isOutput=False)
    XQ = nc.declare_dram_parameter("xq", [QT, D], F32, isOutput=False)
    WQ = nc.declare_dram_parameter("wq", [D, D], MMDT, isOutput=False)
    WK = nc.declare_dram_parameter("wk", [D, D], MMDT, isOutput=False)
    WV = nc.declare_dram_parameter("wv", [D, D], MMDT, isOutput=False)
    WO = nc.declare_dram_parameter("wo", [D, D], MMDT, isOutput=False)
    W1 = nc.declare_dram_parameter("w1", [D, DFF], MMDT, isOutput=False)
    W2 = nc.declare_dram_parameter("w2", [DFF, D], MMDT, isOutput=False)
    BQ = nc.declare_dram_parameter("bq", [D], F32, isOutput=False)
    BK = nc.declare_dram_parameter("bk", [D], F32, isOutput=False)
    BV = nc.declare_dram_parameter("bv", [D], F32, isOutput=False)
    BO = nc.declare_dram_parameter("bo", [D], F32, isOutput=False)
    B1 = nc.declare_dram_parameter("b1", [DFF], F32, isOutput=False)
    B2 = nc.declare_dram_parameter("b2", [D], F32, isOutput=False)
    Y = nc.declare_dram_parameter("y", [QT, D], F32, isOutput=True)

    with TileContext(nc) as tc:
        with (
            tc.tile_pool(name="const", bufs=1) as cpool,
            tc.tile_pool(name="accp", bufs=1) as accp,
            tc.tile_pool(name="x2p", bufs=1) as x2p,
        ):
            ident = cpool.tile([128, 128], F32)
            make_identity(nc, ident)
            eps = cpool.tile([128, 1], F32)
            nc.vector.memset(eps, EPS)
            ones64 = cpool.tile([1, 64], F32)
            nc.vector.memset(ones64, 1.0)
            bqT = cpool.tile([128, 8], F32)
            nc.sync.dma_start(out=bqT, in_=BQ[:].rearrange("(t p) -> p t", p=128))
            bkT = cpool.tile([128, 8], F32)
            nc.sync.dma_start(out=bkT, in_=BK[:].rearrange("(t p) -> p t", p=128))
            b1T = cpool.tile([128, 32], F32)
            nc.sync.dma_start(out=b1T, in_=B1[:].rearrange("(t p) -> p t", p=128))
            bv_bc = cpool.tile([128, D], F32)
            nc.sync.dma_start(out=bv_bc, in_=BV[:].partition_broadcast(128))
            bo_bc = cpool.tile([128, D], F32)
            nc.sync.dma_start(out=bo_bc, in_=BO[:].partition_broadcast(128))
            b2_bc = cpool.tile([128, D], F32)
            nc.sync.dma_start(out=b2_bc, in_=B2[:].partition_broadcast(128))

            acc = accp.tile([65, 16, 512], F32)  # unnormalized attn^T + denom row
            x2 = x2p.tile([128, 4, D], F32)      # post-attention residual stream

            # ---- projections + attention, streamed over kv chunks ----
            with (
                tc.tile_pool(name="qp", bufs=1) as qp,
                tc.tile_pool(name="lnp", bufs=2) as lnp,
                tc.tile_pool(name="hTp", bufs=2) as hTp,
                tc.tile_pool(name="ktp", bufs=1) as ktp,
                tc.tile_pool(name="vp", bufs=2) as vp,
                tc.tile_pool(name="wsm", bufs=4) as wsm,
                tc.tile_pool(name="wvp", bufs=1) as wvp,
                tc.tile_pool(name="pp", bufs=4) as ppl,
                tc.tile_pool(name="psK", bufs=2, space="PSUM") as psK,
                tc.tile_pool(name="psV", bufs=1, space="PSUM") as psV,
                tc.tile_pool(name="psS", bufs=2, space="PSUM") as psS,
                tc.tile_pool(name="psA", bufs=1, space="PSUM") as psA,
                tc.tile_pool(name="psT", bufs=2, space="PSUM") as psT,
            ):
                Q_sb = qp.tile([128, 8, 512], MMDT)  # Q^T [hd, q]

                # Q projection from the core's own tokens
                hqT = hTp.tile([128, 8, 512], MMDT, tag="hT")
                _ln_transpose(nc, lnp, psT, ident, eps, XQ[:], hqT)
                for ht in range(8):
                    psq = psK.tile([128, 512], F32, tag="psK")
                    for dt in range(8):
                        wt = wsm.tile([128, 128], MMDT, tag="w")
                        nc.sync.dma_start(
                            out=wt,
                            in_=WQ[dt * 128:(dt + 1) * 128, ht * 128:(ht + 1) * 128],
                        )
                        nc.tensor.matmul(
                            psq, wt, hqT[:, dt, :], start=(dt == 0), stop=(dt == 7)
                        )
                    nc.vector.tensor_scalar_add(Q_sb[:, ht, :], psq, bqT[:, ht:ht + 1])

                for kc in range(NCHUNK):
                    hT = hTp.tile([128, 8, 512], MMDT, tag="hT")
                    _ln_transpose(nc, lnp, psT, ident, eps, XB[kc * 512:(kc + 1) * 512, :], hT)

                    # K^T chunk [hd, 512]
                    KT = ktp.tile([128, 8, 512], MMDT, tag="KT")
                    for ht in range(8):
                        psk = psK.tile([128, 512], F32, tag="psK")
                        for dt in range(8):
                            wt = wsm.tile([128, 128], MMDT, tag="w")
                            nc.sync.dma_start(
                                out=wt,
                                in_=WK[dt * 128:(dt + 1) * 128, ht * 128:(ht + 1) * 128],
                            )
                            nc.tensor.matmul(
                                psk, wt, hT[:, dt, :], start=(dt == 0), stop=(dt == 7)
                            )
                        nc.vector.tensor_scalar_add(KT[:, ht, :], psk, bkT[:, ht:ht + 1])

                    # V chunk, natural layout [token, head, hd] + ones column
                    V = vp.tile([128, 4, 16, 65], MMDT, tag="V")
                    nc.vector.memset(V[:, :, :, 64:65].bitcast(F32), 1.0)
                    for hc in range(2):
                        wv_sb = wvp.tile([128, 8, 512], MMDT, tag="wv")
                        nc.sync.dma_start(
                            out=wv_sb,
                            in_=WV[:, hc * 512:(hc + 1) * 512].rearrange(
                                "(t p) n -> p t n", p=128
                            ),
                        )
                        for st in range(4):
                            psv = psV.tile([128, 512], F32, tag="psV")
                            for dt in range(8):
                                nc.tensor.matmul(
                                    psv,
                                    hT[:, dt, st * 128:(st + 1) * 128],
                                    wv_sb[:, dt, :],
                                    start=(dt == 0),
                                    stop=(dt == 7),
                                )
                            nc.vector.tensor_add(
                                V[:, st, hc * 8:(hc + 1) * 8, 0:64],
                                psv.rearrange("p (h d) -> p h d", h=8),
                                bv_bc[:, hc * 512:(hc + 1) * 512].rearrange(
                                    "p (h d) -> p h d", h=8
                                ),
                            )

                    # attention for this kv chunk
                    for h in range(H):
                        ko = (h % 2) * 64
                        kj = h // 2
                        p_tiles = []
                        for kt in range(4):
                            pss = psS.tile([128, 512], F32, tag="psS")
                            nc.tensor.matmul(
                                pss,
                                KT[ko:ko + 64, kj, kt * 128:(kt + 1) * 128],
                                Q_sb[ko:ko + 64, kj, :],
                                start=True,
                                stop=True,
                            )
                            P = ppl.tile([128, 512], MMDT, tag="P")
                            nc.scalar.activation(P, pss, AF.Exp, scale=0.125)
                            p_tiles.append(P)
                        psa = psA.tile([65, 512], F32, tag="psA")
                        for kt in range(4):
                            nc.tensor.matmul(
                                psa, V[:, kt, h, :], p_tiles[kt],
                                start=(kt == 0), stop=(kt == 3),
                            )
                        if kc == 0:
                            nc.vector.tensor_copy(acc[:, h, :], psa)
                        else:
                            nc.vector.tensor_add(acc[:, h, :], acc[:, h, :], psa)

            # ---- softmax normalization + out-projection + residual ----
            with (
                tc.tile_pool(name="attnp", bufs=1) as attnp,
                tc.tile_pool(name="dsm", bufs=4) as dsm,
                tc.tile_pool(name="psRB", bufs=2, space="PSUM") as psRB,
                tc.tile_pool(name="xqp", bufs=1) as xqp,
                tc.tile_pool(name="dwo", bufs=3) as dwo,
                tc.tile_pool(name="dtmp", bufs=4) as dtmp,
                tc.tile_pool(name="psO", bufs=4, space="PSUM") as psO,
            ):
                attn64 = attnp.tile([64, 16, 512], MMDT)
                for h in range(H):
                    r = dsm.tile([1, 512], F32, tag="r")
                    nc.vector.reciprocal(r, acc[64:65, h, :])
                    rb_ps = psRB.tile([64, 512], F32, tag="rb")
                    nc.tensor.matmul(rb_ps, ones64, r, start=True, stop=True)
                    rb = dsm.tile([64, 512], F32, tag="rb_sb")
                    nc.scalar.copy(rb, rb_ps)
                    nc.vector.tensor_mul(attn64[:, h, :], acc[0:64, h, :], rb)

                xq_sb = xqp.tile([128, 4, D], F32)
                nc.sync.dma_start(
                    out=xq_sb, in_=XQ[:].rearrange("(t p) n -> p t n", p=128)
                )
                for c in range(2):
                    po = [psO.tile([128, 512], F32, tag="psO", name=f"po{c}_{i}") for i in range(4)]
                    for h in range(H):
                        wot = dwo.tile([64, 512], MMDT, tag="wo")
                        nc.sync.dma_start(
                            out=wot,
                            in_=WO[h * 64:(h + 1) * 64, c * 512:(c + 1) * 512],
                        )
                        for qt in range(4):
                            nc.tensor.matmul(
                                po[qt], attn64[:, h, qt * 128:(qt + 1) * 128], wot,
                                start=(h == 0), stop=(h == H - 1),
                            )
                    for qt in range(4):
                        t1 = dtmp.tile([128, 512], F32, tag="t1")
                        nc.vector.tensor_add(
                            t1, po[qt], bo_bc[:, c * 512:(c + 1) * 512]
                        )
                        nc.vector.tensor_add(
                            x2[:, qt, c * 512:(c + 1) * 512],
                            t1,
                            xq_sb[:, qt, c * 512:(c + 1) * 512],
                        )

            # ---- LN2 + MLP + residual ----
            with (
                tc.tile_pool(name="lnp2", bufs=2) as lnp2,
                tc.tile_pool(name="h2p", bufs=1) as h2p,
                tc.tile_pool(name="gp", bufs=1) as gp,
                tc.tile_pool(name="wfp", bufs=3) as wfp,
                tc.tile_pool(name="yp", bufs=2) as yp,
                tc.tile_pool(name="psT2", bufs=2, space="PSUM") as psT2,
                tc.tile_pool(name="psF", bufs=2, space="PSUM") as psF,
                tc.tile_pool(name="psY", bufs=4, space="PSUM") as psY,
            ):
                h2T = h2p.tile([128, 8, 512], MMDT)
                _ln_transpose(nc, lnp2, psT2, ident, eps, x2, h2T, from_sbuf=True)

                # MLP1: gelu(h2 @ w1 + b1), transposed output [dff, q]
                G = gp.tile([128, 32, 512], MMDT)
                for ft in range(32):
                    psf = psF.tile([128, 512], F32, tag="psF")
                    for dt in range(8):
                        w1t = wfp.tile([128, 128], MMDT, tag="w1")
                        nc.sync.dma_start(
                            out=w1t,
                            in_=W1[dt * 128:(dt + 1) * 128, ft * 128:(ft + 1) * 128],
                        )
                        nc.tensor.matmul(
                            psf, w1t, h2T[:, dt, :], start=(dt == 0), stop=(dt == 7)
                        )
                    nc.scalar.activation(
                        G[:, ft, :], psf, AF.Gelu, bias=b1T[:, ft:ft + 1]
                    )

                # MLP2: y = G^T @ w2 + b2 + x2
                for c in range(2):
                    py = [psY.tile([128, 512], F32, tag="psY", name=f"py{c}_{i}") for i in range(4)]
                    for ft in range(32):
                        w2t = wfp.tile([128, 512], MMDT, tag="w2")
                        nc.sync.dma_start(
                            out=w2t,
                            in_=W2[ft * 128:(ft + 1) * 128, c * 512:(c + 1) * 512],
                        )
                        for qt in range(4):
                            nc.tensor.matmul(
                                py[qt], G[:, ft, qt * 128:(qt + 1) * 128], w2t,
                                start=(ft == 0), stop=(ft == 31),
                            )
                    for qt in range(4):
                        t1 = yp.tile([128, 512], F32, tag="yt1")
                        nc.vector.tensor_add(
                            t1, py[qt], b2_bc[:, c * 512:(c + 1) * 512]
                        )
                        yt = yp.tile([128, 512], F32, tag="yt2")
                        nc.vector.tensor_add(
                            yt, t1, x2[:, qt, c * 512:(c + 1) * 512]
                        )
                        nc.sync.dma_start(
                            out=Y[qt * 128:(qt + 1) * 128, c * 512:(c + 1) * 512],
                            in_=yt,
                        )

    nc.compile()
    return nc


_NC = None


def _get_nc():
    global _NC
    if _NC is None:
        _NC = _build()
    return _NC


def kernel(x, ln1_g, ln1_b, wq, bq, wk, bk, wv, bv, wo, bo, w1, b1, w2, b2, ln2_g, ln2_b):
    f32 = lambda a: np.ascontiguousarray(np.asarray(a, dtype=np.float32))
    x = f32(x)
    ln1_g, ln1_b = f32(ln1_g), f32(ln1_b)
    ln2_g, ln2_b = f32(ln2_g), f32(ln2_b)
    wq, wk, wv, wo = f32(wq), f32(wk), f32(wv), f32(wo)
    w1, w2 = f32(w1), f32(w2)
    bq, bk, bv, bo, b1, b2 = f32(bq), f32(bk), f32(bv), f32(bo), f32(b1), f32(b2)

    # Fold LayerNorm affine params into the following projections (exact).
    wq_e = f32(ln1_g[:, None] * wq)
    wk_e = f32(ln1_g[:, None] * wk)
    wv_e = f32(ln1_g[:, None] * wv)
    bq_e = f32(bq + ln1_b @ wq)
    bk_e = f32(bk + ln1_b @ wk)
    bv_e = f32(bv + ln1_b @ wv)
    w1_e = f32(ln2_g[:, None] * w1)
    b1_e = f32(b1 + ln2_b @ w1)

    common = {
        "wq": wq_e, "wk": wk_e, "wv": wv_e, "wo": wo,
        "w1": w1_e, "w2": w2,
        "bq": bq_e, "bk": bk_e, "bv": bv_e, "bo": bo,
        "b1": b1_e, "b2": b2,
    }
    in_maps = []
    for c in range(NCORES):
        b = c // 4
        qoff = (c % 4) * QT
        m = dict(common)
        m["xb"] = np.ascontiguousarray(x[b])
        m["xq"] = np.ascontiguousarray(x[b, qoff:qoff + QT])
        in_maps.append(m)

    nc = _get_nc()
    res = run_bass_kernel_spmd(nc, in_maps, core_ids=list(range(NCORES)))

    y = np.empty((B, S, D), dtype=np.float32)
    for c in range(NCORES):
        b = c // 4
        qoff = (c % 4) * QT
        y[b, qoff:qoff + QT] = res.results[c]["y"]
    return y


# revision 11
# speedup vs baseline: 2.0513x; 1.1129x over previous
"""Transformer encoder layer (LN -> MHA -> residual -> LN -> MLP -> residual)
on 8 Trainium2 NeuronCores.

Sharding: token-parallel over the 4096 (batch*seq) tokens, 512 query-tokens
per core; the 4 cores sharing a batch each redundantly compute the full
2048-token K/V for that batch, so no collectives are needed.

On-chip layout: activations are kept feature-major ("transposed", [d, token])
so every matmul contracts along the partition dim with weights in natural
[d_in, d_out] layout.  Softmax is computed unnormalized (scores are bounded,
so plain exp is numerically safe and algebraically identical); the denominator
comes for free from a ones-column appended to V, and the division is applied
in place to the tiny per-head attention accumulator.

LayerNorm gains/biases are folded into the following projections on the host
(exact algebra: (g*xhat+b) @ W = xhat @ (diag(g) W) + b @ W).
"""

import numpy as np

import concourse.bass as bass
import concourse.mybir as mybir
from concourse import bacc
from concourse.tile import TileContext
from concourse.bass_utils import run_bass_kernel_spmd
from concourse.masks import make_identity

F32 = mybir.dt.float32
F32R = mybir.dt.float32r
MMDT = F32R  # dtype for matmul operands (float32r = full-rate PE)
AF = mybir.ActivationFunctionType
ALU = mybir.AluOpType

B, S, D = 2, 2048, 1024
H, HD = 16, 64
DFF = 4 * D
NCORES = 8
QT = 512           # query tokens per core
NCHUNK = S // 512  # kv chunks of 512 tokens
EPS = 1e-5


def _ln_transpose(nc, lnp, psT, ident, eps, x_src, hT, from_sbuf=False):
    """LayerNorm 512 tokens and write the transposed [d, token] result into
    hT ([128, 8, 512]).  x_src: DRAM AP rows [512, D] or SBUF tile view
    [128, 4, D]."""
    for st in range(4):
        if from_sbuf:
            xt = x_src[:, st, :]
        else:
            xt = lnp.tile([128, D], F32, tag="ln_x")
            nc.sync.dma_start(out=xt, in_=x_src[st * 128:(st + 1) * 128, :])
        stats = lnp.tile([128, 2, 6], F32, tag="ln_st")
        nc.vector.bn_stats(stats[:, 0, :], xt[:, 0:512])
        nc.vector.bn_stats(stats[:, 1, :], xt[:, 512:1024])
        mv = lnp.tile([128, 2], F32, tag="ln_mv")
        nc.vector.bn_aggr(mv, stats)
        sd = lnp.tile([128, 1], F32, tag="ln_sd")
        nc.scalar.activation(sd, mv[:, 1:2], AF.Sqrt, bias=eps[:, 0:1])
        rstd = lnp.tile([128, 1], F32, tag="ln_rs")
        nc.vector.reciprocal(rstd, sd)
        h = lnp.tile([128, D], F32, tag="ln_h")
        nc.vector.tensor_scalar(h, xt, mv[:, 0:1], rstd[:, 0:1], ALU.subtract, ALU.mult)
        for dt in range(8):
            pst = psT.tile([128, 128], F32, tag="tp")
            nc.tensor.transpose(pst, h[:, dt * 128:(dt + 1) * 128], ident)
            nc.vector.tensor_copy(hT[:, dt, st * 128:(st + 1) * 128], pst)


def _build():
    nc = bacc.Bacc(None, target_bir_lowering=False)

    XB = nc.declare_dram_parameter("xb", [S, D], F32, isOutput=False)
    XQ = nc.declare_dram_parameter("xq", [QT, D], F32, isOutput=False)
    WQ = nc.declare_dram_parameter("wq", [D, D], MMDT, isOutput=False)
    WK = nc.declare_dram_parameter("wk", [D, D], MMDT, isOutput=False)
    WV = nc.declare_dram_parameter("wv", [D, D], MMDT, isOutput=False)
    WO = nc.declare_dram_parameter("wo", [D, D], MMDT, isOutput=False)
    W1 = nc.declare_dram_parameter("w1", [D, DFF], MMDT, isOutput=False)
    W2 = nc.declare_dram_parameter("w2", [DFF, D], MMDT, isOutput=False)
    BQ = nc.declare_dram_parameter("bq", [D], F32, isOutput=False)
    BK = nc.declare_dram_parameter("bk", [D], F32, isOutput=False)
    BV = nc.declare_dram_parameter("bv", [D], F32, isOutput=False)
    BO = nc.declare_dram_parameter("bo", [D], F32, isOutput=False)
    B1 = nc.declare_dram_parameter("b1", [DFF], F32, isOutput=False)
    B2 = nc.declare_dram_parameter("b2", [D], F32, isOutput=False)
    Y = nc.declare_dram_parameter("y", [QT, D], F32, isOutput=True)

    with TileContext(nc) as tc:
        with (
            tc.tile_pool(name="const", bufs=1) as cpool,
            tc.tile_pool(name="accp", bufs=1) as accp,
            tc.tile_pool(name="x2p", bufs=1) as x2p,
        ):
            ident = cpool.tile([128, 128], F32)
            make_identity(nc, ident)
            eps = cpool.tile([128, 1], F32)
            nc.vector.memset(eps, EPS)
            ones64 = cpool.tile([1, 64], F32)
            nc.vector.memset(ones64, 1.0)
            bqT = cpool.tile([128, 8], F32)
            nc.sync.dma_start(out=bqT, in_=BQ[:].rearrange("(t p) -> p t", p=128))
            bkT = cpool.tile([128, 8], F32)
            nc.sync.dma_start(out=bkT, in_=BK[:].rearrange("(t p) -> p t", p=128))
            b1T = cpool.tile([128, 32], F32)
            nc.sync.dma_start(out=b1T, in_=B1[:].rearrange("(t p) -> p t", p=128))
            bv_bc = cpool.tile([128, D], F32)
            nc.sync.dma_start(out=bv_bc, in_=BV[:].partition_broadcast(128))
            bo_bc = cpool.tile([128, D], F32)
            nc.sync.dma_start(out=bo_bc, in_=BO[:].partition_broadcast(128))
            b2_bc = cpool.tile([128, D], F32)
            nc.sync.dma_start(out=b2_bc, in_=B2[:].partition_broadcast(128))

            acc = accp.tile([65, 16, 512], F32)  # unnormalized attn^T + denom row
            x2 = x2p.tile([128, 4, D], F32)      # post-attention residual stream

            # ---- projections + attention, streamed over kv chunks ----
            with (
                tc.tile_pool(name="qp", bufs=1) as qp,
                tc.tile_pool(name="lnp", bufs=2) as lnp,
                tc.tile_pool(name="hTp", bufs=2) as hTp,
                tc.tile_pool(name="ktp", bufs=1) as ktp,
                tc.tile_pool(name="vp", bufs=2) as vp,
                tc.tile_pool(name="wsm", bufs=8) as wsm,
                tc.tile_pool(name="wvp", bufs=1) as wvp,
                tc.tile_pool(name="pp", bufs=6) as ppl,
                tc.tile_pool(name="psK", bufs=2, space="PSUM") as psK,
                tc.tile_pool(name="psV", bufs=1, space="PSUM") as psV,
                tc.tile_pool(name="psS", bufs=2, space="PSUM") as psS,
                tc.tile_pool(name="psA", bufs=1, space="PSUM") as psA,
                tc.tile_pool(name="psT", bufs=2, space="PSUM") as psT,
            ):
                Q_sb = qp.tile([128, 8, 512], MMDT)  # Q^T [hd, q]

                # Q projection from the core's own tokens
                hqT = hTp.tile([128, 8, 512], MMDT, tag="hT")
                _ln_transpose(nc, lnp, psT, ident, eps, XQ[:], hqT)
                for ht in range(8):
                    psq = psK.tile([128, 512], F32, tag="psK")
                    for dt in range(8):
                        wt = wsm.tile([128, 128], MMDT, tag="w")
                        nc.sync.dma_start(
                            out=wt,
                            in_=WQ[dt * 128:(dt + 1) * 128, ht * 128:(ht + 1) * 128],
                        )
                        nc.tensor.matmul(
                            psq, wt, hqT[:, dt, :], start=(dt == 0), stop=(dt == 7)
                        )
                    nc.vector.tensor_scalar_add(Q_sb[:, ht, :], psq, bqT[:, ht:ht + 1])

                for kc in range(NCHUNK):
                    hT = hTp.tile([128, 8, 512], MMDT, tag="hT")
                    _ln_transpose(nc, lnp, psT, ident, eps, XB[kc * 512:(kc + 1) * 512, :], hT)

                    # K^T chunk [hd, 512]
                    KT = ktp.tile([128, 8, 512], MMDT, tag="KT")
                    for ht in range(8):
                        psk = psK.tile([128, 512], F32, tag="psK")
                        for dt in range(8):
                            wt = wsm.tile([128, 128], MMDT, tag="w")
                            nc.sync.dma_start(
                                out=wt,
                                in_=WK[dt * 128:(dt + 1) * 128, ht * 128:(ht + 1) * 128],
                            )
                            nc.tensor.matmul(
                                psk, wt, hT[:, dt, :], start=(dt == 0), stop=(dt == 7)
                            )
                        nc.vector.tensor_scalar_add(KT[:, ht, :], psk, bkT[:, ht:ht + 1])

                    # V chunk, natural layout [token, head, hd] + ones column
                    V = vp.tile([128, 4, 16, 65], MMDT, tag="V")
                    nc.vector.memset(V[:, :, :, 64:65].bitcast(F32), 1.0)
                    for hc in range(2):
                        wv_sb = wvp.tile([128, 8, 512], MMDT, tag="wv")
                        nc.sync.dma_start(
                            out=wv_sb,
                            in_=WV[:, hc * 512:(hc + 1) * 512].rearrange(
                                "(t p) n -> p t n", p=128
                            ),
                        )
                        for st in range(4):
                            psv = psV.tile([128, 512], F32, tag="psV")
                            for dt in range(8):
                                nc.tensor.matmul(
                                    psv,
                                    hT[:, dt, st * 128:(st + 1) * 128],
                                    wv_sb[:, dt, :],
                                    start=(dt == 0),
                                    stop=(dt == 7),
                                )
                            nc.vector.tensor_add(
                                V[:, st, hc * 8:(hc + 1) * 8, 0:64],
                                psv.rearrange("p (h d) -> p h d", h=8),
                                bv_bc[:, hc * 512:(hc + 1) * 512].rearrange(
                                    "p (h d) -> p h d", h=8
                                ),
                            )

                    # attention for this kv chunk
                    for h in range(H):
                        ko = (h % 2) * 64
                        kj = h // 2
                        p_tiles = []
                        for kt in range(4):
                            pss = psS.tile([128, 512], F32, tag="psS")
                            nc.tensor.matmul(
                                pss,
                                KT[ko:ko + 64, kj, kt * 128:(kt + 1) * 128],
                                Q_sb[ko:ko + 64, kj, :],
                                start=True,
                                stop=True,
                            )
                            P = ppl.tile([128, 512], MMDT, tag="P")
                            nc.scalar.activation(P, pss, AF.Exp, scale=0.125)
                            p_tiles.append(P)
                        psa = psA.tile([65, 512], F32, tag="psA")
                        for kt in range(4):
                            nc.tensor.matmul(
                                psa, V[:, kt, h, :], p_tiles[kt],
                                start=(kt == 0), stop=(kt == 3),
                            )
                        if kc == 0:
                            nc.vector.tensor_copy(acc[:, h, :], psa)
                        else:
                            nc.vector.tensor_add(acc[:, h, :], acc[:, h, :], psa)

            # ---- softmax normalization + out-projection + residual ----
            with (
                tc.tile_pool(name="attnp", bufs=1) as attnp,
                tc.tile_pool(name="dsm", bufs=4) as dsm,
                tc.tile_pool(name="psRB", bufs=2, space="PSUM") as psRB,
                tc.tile_pool(name="xqp", bufs=1) as xqp,
                tc.tile_pool(name="dwo", bufs=6) as dwo,
                tc.tile_pool(name="dtmp", bufs=4) as dtmp,
                tc.tile_pool(name="psO", bufs=4, space="PSUM") as psO,
            ):
                attn64 = attnp.tile([64, 16, 512], MMDT)
                for h in range(H):
                    r = dsm.tile([1, 512], F32, tag="r")
                    nc.vector.reciprocal(r, acc[64:65, h, :])
                    rb_ps = psRB.tile([64, 512], F32, tag="rb")
                    nc.tensor.matmul(rb_ps, ones64, r, start=True, stop=True)
                    rb = dsm.tile([64, 512], F32, tag="rb_sb")
                    nc.scalar.copy(rb, rb_ps)
                    nc.vector.tensor_mul(attn64[:, h, :], acc[0:64, h, :], rb)

                xq_sb = xqp.tile([128, 4, D], F32)
                nc.sync.dma_start(
                    out=xq_sb, in_=XQ[:].rearrange("(t p) n -> p t n", p=128)
                )
                for c in range(2):
                    po = [psO.tile([128, 512], F32, tag="psO", name=f"po{c}_{i}") for i in range(4)]
                    for h in range(H):
                        wot = dwo.tile([64, 512], MMDT, tag="wo")
                        nc.sync.dma_start(
                            out=wot,
                            in_=WO[h * 64:(h + 1) * 64, c * 512:(c + 1) * 512],
                        )
                        for qt in range(4):
                            nc.tensor.matmul(
                                po[qt], attn64[:, h, qt * 128:(qt + 1) * 128], wot,
                                start=(h == 0), stop=(h == H - 1),
                            )
                    for qt in range(4):
                        t1 = dtmp.tile([128, 512], F32, tag="t1")
                        nc.vector.tensor_add(
                            t1, po[qt], bo_bc[:, c * 512:(c + 1) * 512]
                        )
                        nc.vector.tensor_add(
                            x2[:, qt, c * 512:(c + 1) * 512],
                            t1,
                            xq_sb[:, qt, c * 512:(c + 1) * 512],
                        )

            # ---- LN2 + MLP + residual ----
            with (
                tc.tile_pool(name="lnp2", bufs=2) as lnp2,
                tc.tile_pool(name="h2p", bufs=1) as h2p,
                tc.tile_pool(name="gp", bufs=1) as gp,
                tc.tile_pool(name="wfp", bufs=12) as wfp,
                tc.tile_pool(name="w2p", bufs=6) as w2p,
                tc.tile_pool(name="yp", bufs=2) as yp,
            ):
                h2T = h2p.tile([128, 8, 512], MMDT)
                G = gp.tile([128, 32, 512], MMDT)
                with (
                    tc.tile_pool(name="psT2", bufs=2, space="PSUM") as psT2,
                    tc.tile_pool(name="psF", bufs=4, space="PSUM") as psF,
                ):
                    _ln_transpose(nc, lnp2, psT2, ident, eps, x2, h2T, from_sbuf=True)

                    # MLP1: gelu(h2 @ w1 + b1), transposed output [dff, q]
                    for ft in range(32):
                        psf = psF.tile([128, 512], F32, tag="psF")
                        for dt in range(8):
                            w1t = wfp.tile([128, 128], MMDT, tag="w1")
                            nc.sync.dma_start(
                                out=w1t,
                                in_=W1[dt * 128:(dt + 1) * 128, ft * 128:(ft + 1) * 128],
                            )
                            nc.tensor.matmul(
                                psf, w1t, h2T[:, dt, :], start=(dt == 0), stop=(dt == 7)
                            )
                        nc.scalar.activation(
                            G[:, ft, :], psf, AF.Gelu, bias=b1T[:, ft:ft + 1]
                        )

                # MLP2: y = G^T @ w2 + b2 + x2
                with tc.tile_pool(name="psY", bufs=4, space="PSUM") as psY:
                  for c in range(2):
                    py = [psY.tile([128, 512], F32, tag="psY", name=f"py{c}_{i}") for i in range(4)]
                    for ft in range(32):
                        w2t = w2p.tile([128, 512], MMDT, tag="w2")
                        nc.sync.dma_start(
                            out=w2t,
                            in_=W2[ft * 128:(ft + 1) * 128, c * 512:(c + 1) * 512],
                        )
                        for qt in range(4):
                            nc.tensor.matmul(
                                py[qt], G[:, ft, qt * 128:(qt + 1) * 128], w2t,
                                start=(ft == 0), stop=(ft == 31),
                            )
                    for qt in range(4):
                        t1 = yp.tile([128, 512], F32, tag="yt1")
                        nc.vector.tensor_add(
                            t1, py[qt], b2_bc[:, c * 512:(c + 1) * 512]
                        )
                        yt = yp.tile([128, 512], F32, tag="yt2")
                        nc.vector.tensor_add(
                            yt, t1, x2[:, qt, c * 512:(c + 1) * 512]
                        )
                        nc.sync.dma_start(
                            out=Y[qt * 128:(qt + 1) * 128, c * 512:(c + 1) * 512],
                            in_=yt,
                        )

    nc.compile()
    return nc


_NC = None


def _get_nc():
    global _NC
    if _NC is None:
        _NC = _build()
    return _NC


def kernel(x, ln1_g, ln1_b, wq, bq, wk, bk, wv, bv, wo, bo, w1, b1, w2, b2, ln2_g, ln2_b):
    f32 = lambda a: np.ascontiguousarray(np.asarray(a, dtype=np.float32))
    x = f32(x)
    ln1_g, ln1_b = f32(ln1_g), f32(ln1_b)
    ln2_g, ln2_b = f32(ln2_g), f32(ln2_b)
    wq, wk, wv, wo = f32(wq), f32(wk), f32(wv), f32(wo)
    w1, w2 = f32(w1), f32(w2)
    bq, bk, bv, bo, b1, b2 = f32(bq), f32(bk), f32(bv), f32(bo), f32(b1), f32(b2)

    # Fold LayerNorm affine params into the following projections (exact).
    wq_e = f32(ln1_g[:, None] * wq)
    wk_e = f32(ln1_g[:, None] * wk)
    wv_e = f32(ln1_g[:, None] * wv)
    bq_e = f32(bq + ln1_b @ wq)
    bk_e = f32(bk + ln1_b @ wk)
    bv_e = f32(bv + ln1_b @ wv)
    w1_e = f32(ln2_g[:, None] * w1)
    b1_e = f32(b1 + ln2_b @ w1)

    common = {
        "wq": wq_e, "wk": wk_e, "wv": wv_e, "wo": wo,
        "w1": w1_e, "w2": w2,
        "bq": bq_e, "bk": bk_e, "bv": bv_e, "bo": bo,
        "b1": b1_e, "b2": b2,
    }
    in_maps = []
    for c in range(NCORES):
        b = c // 4
        qoff = (c % 4) * QT
        m = dict(common)
        m["xb"] = np.ascontiguousarray(x[b])
        m["xq"] = np.ascontiguousarray(x[b, qoff:qoff + QT])
        in_maps.append(m)

    nc = _get_nc()
    res = run_bass_kernel_spmd(nc, in_maps, core_ids=list(range(NCORES)))

    y = np.empty((B, S, D), dtype=np.float32)
    for c in range(NCORES):
        b = c // 4
        qoff = (c % 4) * QT
        y[b, qoff:qoff + QT] = res.results[c]["y"]
    return y


# revision 13
# speedup vs baseline: 2.6462x; 1.2900x over previous
"""Transformer encoder layer (LN -> MHA -> residual -> LN -> MLP -> residual)
on 8 Trainium2 NeuronCores.

Sharding: token-parallel over the 4096 (batch*seq) tokens, 512 query-tokens
per core; the 4 cores sharing a batch each redundantly compute the full
2048-token K/V for that batch, so no collectives are needed.

On-chip layout: activations are kept feature-major ("transposed", [d, token])
so every matmul contracts along the partition dim with weights in natural
[d_in, d_out] layout.  Softmax is computed unnormalized (scores are bounded,
so plain exp is numerically safe and algebraically identical); the denominator
comes for free from a ones-column appended to V, and the division is applied
in place to the tiny per-head attention accumulator.

LayerNorm gains/biases are folded into the following projections on the host
(exact algebra: (g*xhat+b) @ W = xhat @ (diag(g) W) + b @ W).
"""

import numpy as np

import concourse.bass as bass
import concourse.mybir as mybir
from concourse import bacc
from concourse.tile import TileContext
from concourse.bass_utils import run_bass_kernel_spmd
from concourse.masks import make_identity

F32 = mybir.dt.float32
F32R = mybir.dt.float32r
MMDT = F32R  # dtype for matmul operands (float32r = full-rate PE)
AF = mybir.ActivationFunctionType
ALU = mybir.AluOpType

B, S, D = 2, 2048, 1024
H, HD = 16, 64
DFF = 4 * D
NCORES = 8
QT = 512           # query tokens per core
NCHUNK = S // 512  # kv chunks of 512 tokens
EPS = 1e-5


def _ln_transpose(nc, lnp, psT, ident, eps, x_src, hT, from_sbuf=False):
    """LayerNorm 512 tokens and write the transposed [d, token] result into
    hT ([128, 8, 512]).  x_src: DRAM AP rows [512, D] or SBUF tile view
    [128, 4, D]."""
    for st in range(4):
        if from_sbuf:
            xt = x_src[:, st, :]
        else:
            xt = lnp.tile([128, D], F32, tag="ln_x")
            nc.sync.dma_start(out=xt, in_=x_src[st * 128:(st + 1) * 128, :])
        stats = lnp.tile([128, 2, 6], F32, tag="ln_st")
        nc.vector.bn_stats(stats[:, 0, :], xt[:, 0:512])
        nc.vector.bn_stats(stats[:, 1, :], xt[:, 512:1024])
        mv = lnp.tile([128, 2], F32, tag="ln_mv")
        nc.vector.bn_aggr(mv, stats)
        sd = lnp.tile([128, 1], F32, tag="ln_sd")
        nc.scalar.activation(sd, mv[:, 1:2], AF.Sqrt, bias=eps[:, 0:1])
        rstd = lnp.tile([128, 1], F32, tag="ln_rs")
        nc.vector.reciprocal(rstd, sd)
        h = lnp.tile([128, D], F32, tag="ln_h")
        nc.vector.tensor_scalar(h, xt, mv[:, 0:1], rstd[:, 0:1], ALU.subtract, ALU.mult)
        for dt in range(8):
            pst = psT.tile([128, 128], F32, tag="tp")
            nc.tensor.transpose(pst, h[:, dt * 128:(dt + 1) * 128], ident)
            nc.vector.tensor_copy(hT[:, dt, st * 128:(st + 1) * 128], pst)


def _build():
    nc = bacc.Bacc(None, target_bir_lowering=False)

    XB = nc.declare_dram_parameter("xb", [S, D], F32, isOutput=False)
    XQ = nc.declare_dram_parameter("xq", [QT, D], F32, isOutput=False)
    WQ = nc.declare_dram_parameter("wq", [D, D], MMDT, isOutput=False)
    WK = nc.declare_dram_parameter("wk", [D, D], MMDT, isOutput=False)
    WV = nc.declare_dram_parameter("wv", [D, D], MMDT, isOutput=False)
    WO = nc.declare_dram_parameter("wo", [D, D], MMDT, isOutput=False)
    W1 = nc.declare_dram_parameter("w1", [D, DFF], MMDT, isOutput=False)
    W2 = nc.declare_dram_parameter("w2", [DFF, D], MMDT, isOutput=False)
    BQ = nc.declare_dram_parameter("bq", [D], F32, isOutput=False)
    BK = nc.declare_dram_parameter("bk", [D], F32, isOutput=False)
    BV = nc.declare_dram_parameter("bv", [D], F32, isOutput=False)
    BO = nc.declare_dram_parameter("bo", [D], F32, isOutput=False)
    B1 = nc.declare_dram_parameter("b1", [DFF], F32, isOutput=False)
    B2 = nc.declare_dram_parameter("b2", [D], F32, isOutput=False)
    Y = nc.declare_dram_parameter("y", [QT, D], F32, isOutput=True)

    with TileContext(nc) as tc:
        with (
            tc.tile_pool(name="const", bufs=1) as cpool,
            tc.tile_pool(name="accp", bufs=1) as accp,
            tc.tile_pool(name="x2p", bufs=1) as x2p,
        ):
            ident = cpool.tile([128, 128], F32)
            make_identity(nc, ident)
            eps = cpool.tile([128, 1], F32)
            nc.vector.memset(eps, EPS)
            ones64 = cpool.tile([1, 64], F32)
            nc.vector.memset(ones64, 1.0)
            bqT = cpool.tile([128, 8], F32)
            nc.sync.dma_start(out=bqT, in_=BQ[:].rearrange("(t p) -> p t", p=128))
            bkT = cpool.tile([128, 8], F32)
            nc.sync.dma_start(out=bkT, in_=BK[:].rearrange("(t p) -> p t", p=128))
            b1T = cpool.tile([128, 32], F32)
            nc.sync.dma_start(out=b1T, in_=B1[:].rearrange("(t p) -> p t", p=128))
            bv_bc = cpool.tile([128, D], F32)
            nc.sync.dma_start(out=bv_bc, in_=BV[:].partition_broadcast(128))
            bo_bc = cpool.tile([128, D], F32)
            nc.sync.dma_start(out=bo_bc, in_=BO[:].partition_broadcast(128))
            b2_bc = cpool.tile([128, D], F32)
            nc.sync.dma_start(out=b2_bc, in_=B2[:].partition_broadcast(128))

            acc = accp.tile([65, 16, 512], F32)  # unnormalized attn^T + denom row
            x2 = x2p.tile([128, 4, D], F32)      # post-attention residual stream

            # ---- projections + attention, streamed over kv chunks ----
            with (
                tc.tile_pool(name="qp", bufs=1) as qp,
                tc.tile_pool(name="lnp", bufs=2) as lnp,
                tc.tile_pool(name="hTp", bufs=2) as hTp,
                tc.tile_pool(name="ktp", bufs=1) as ktp,
                tc.tile_pool(name="vp", bufs=2) as vp,
                tc.tile_pool(name="wsm", bufs=2) as wsm,
                tc.tile_pool(name="wvp", bufs=1) as wvp,
                tc.tile_pool(name="pp", bufs=4) as ppl,
                tc.tile_pool(name="psK", bufs=2, space="PSUM") as psK,
                tc.tile_pool(name="psV", bufs=1, space="PSUM") as psV,
                tc.tile_pool(name="psS", bufs=2, space="PSUM") as psS,
                tc.tile_pool(name="psA", bufs=1, space="PSUM") as psA,
                tc.tile_pool(name="psT", bufs=2, space="PSUM") as psT,
            ):
                Q_sb = qp.tile([128, 8, 512], MMDT)  # Q^T [hd, q]

                # Q projection from the core's own tokens
                hqT = hTp.tile([128, 8, 512], MMDT, tag="hT")
                _ln_transpose(nc, lnp, psT, ident, eps, XQ[:], hqT)
                for ht in range(8):
                    wcol = wsm.tile([128, 8, 128], MMDT, tag="w")
                    nc.sync.dma_start(
                        out=wcol,
                        in_=WQ[:, ht * 128:(ht + 1) * 128].rearrange(
                            "(t p) n -> p t n", p=128
                        ),
                    )
                    psq = psK.tile([128, 512], F32, tag="psK")
                    for dt in range(8):
                        nc.tensor.matmul(
                            psq, wcol[:, dt, :], hqT[:, dt, :],
                            start=(dt == 0), stop=(dt == 7),
                        )
                    nc.vector.tensor_scalar_add(Q_sb[:, ht, :], psq, bqT[:, ht:ht + 1])

                for kc in range(NCHUNK):
                    hT = hTp.tile([128, 8, 512], MMDT, tag="hT")
                    _ln_transpose(nc, lnp, psT, ident, eps, XB[kc * 512:(kc + 1) * 512, :], hT)

                    # K^T chunk [hd, 512]
                    KT = ktp.tile([128, 8, 512], MMDT, tag="KT")
                    for ht in range(8):
                        wcol = wsm.tile([128, 8, 128], MMDT, tag="w")
                        nc.sync.dma_start(
                            out=wcol,
                            in_=WK[:, ht * 128:(ht + 1) * 128].rearrange(
                                "(t p) n -> p t n", p=128
                            ),
                        )
                        psk = psK.tile([128, 512], F32, tag="psK")
                        for dt in range(8):
                            nc.tensor.matmul(
                                psk, wcol[:, dt, :], hT[:, dt, :],
                                start=(dt == 0), stop=(dt == 7),
                            )
                        nc.vector.tensor_scalar_add(KT[:, ht, :], psk, bkT[:, ht:ht + 1])

                    # V chunk, natural layout [token, head, hd] + ones column
                    V = vp.tile([128, 4, 16, 65], MMDT, tag="V")
                    nc.vector.memset(V[:, :, :, 64:65].bitcast(F32), 1.0)
                    for hc in range(2):
                        wv_sb = wvp.tile([128, 8, 512], MMDT, tag="wv")
                        nc.sync.dma_start(
                            out=wv_sb,
                            in_=WV[:, hc * 512:(hc + 1) * 512].rearrange(
                                "(t p) n -> p t n", p=128
                            ),
                        )
                        for st in range(4):
                            psv = psV.tile([128, 512], F32, tag="psV")
                            for dt in range(8):
                                nc.tensor.matmul(
                                    psv,
                                    hT[:, dt, st * 128:(st + 1) * 128],
                                    wv_sb[:, dt, :],
                                    start=(dt == 0),
                                    stop=(dt == 7),
                                )
                            nc.vector.tensor_add(
                                V[:, st, hc * 8:(hc + 1) * 8, 0:64],
                                psv.rearrange("p (h d) -> p h d", h=8),
                                bv_bc[:, hc * 512:(hc + 1) * 512].rearrange(
                                    "p (h d) -> p h d", h=8
                                ),
                            )

                    # attention for this kv chunk
                    for h in range(H):
                        ko = (h % 2) * 64
                        kj = h // 2
                        p_tiles = []
                        for kt in range(4):
                            pss = psS.tile([128, 512], F32, tag="psS")
                            nc.tensor.matmul(
                                pss,
                                KT[ko:ko + 64, kj, kt * 128:(kt + 1) * 128],
                                Q_sb[ko:ko + 64, kj, :],
                                start=True,
                                stop=True,
                            )
                            P = ppl.tile([128, 512], MMDT, tag="P")
                            nc.scalar.activation(P, pss, AF.Exp, scale=0.125)
                            p_tiles.append(P)
                        psa = psA.tile([65, 512], F32, tag="psA")
                        for kt in range(4):
                            nc.tensor.matmul(
                                psa, V[:, kt, h, :], p_tiles[kt],
                                start=(kt == 0), stop=(kt == 3),
                            )
                        if kc == 0:
                            nc.vector.tensor_copy(acc[:, h, :], psa)
                        else:
                            nc.vector.tensor_add(acc[:, h, :], acc[:, h, :], psa)

            # ---- softmax normalization + out-projection + residual ----
            with (
                tc.tile_pool(name="attnp", bufs=1) as attnp,
                tc.tile_pool(name="dsm", bufs=4) as dsm,
                tc.tile_pool(name="psRB", bufs=2, space="PSUM") as psRB,
                tc.tile_pool(name="xqp", bufs=1) as xqp,
                tc.tile_pool(name="dwo", bufs=6) as dwo,
                tc.tile_pool(name="dtmp", bufs=4) as dtmp,
                tc.tile_pool(name="psO", bufs=4, space="PSUM") as psO,
            ):
                attn128 = attnp.tile([128, 8, 512], MMDT)
                for h in range(H):
                    r = dsm.tile([1, 512], F32, tag="r")
                    nc.vector.reciprocal(r, acc[64:65, h, :])
                    rb_ps = psRB.tile([64, 512], F32, tag="rb")
                    nc.tensor.matmul(rb_ps, ones64, r, start=True, stop=True)
                    rb = dsm.tile([64, 512], F32, tag="rb_sb")
                    nc.scalar.copy(rb, rb_ps)
                    ko = (h % 2) * 64
                    nc.vector.tensor_mul(
                        attn128[ko:ko + 64, h // 2, :], acc[0:64, h, :], rb
                    )

                xq_sb = xqp.tile([128, 4, D], F32)
                nc.sync.dma_start(
                    out=xq_sb, in_=XQ[:].rearrange("(t p) n -> p t n", p=128)
                )
                for c in range(2):
                    po = [psO.tile([128, 512], F32, tag="psO", name=f"po{c}_{i}") for i in range(4)]
                    for j in range(8):
                        wot = dwo.tile([128, 512], MMDT, tag="wo")
                        nc.sync.dma_start(
                            out=wot,
                            in_=WO[j * 128:(j + 1) * 128, c * 512:(c + 1) * 512],
                        )
                        for qt in range(4):
                            nc.tensor.matmul(
                                po[qt], attn128[:, j, qt * 128:(qt + 1) * 128], wot,
                                start=(j == 0), stop=(j == 7),
                            )
                    for qt in range(4):
                        t1 = dtmp.tile([128, 512], F32, tag="t1")
                        nc.vector.tensor_add(
                            t1, po[qt], bo_bc[:, c * 512:(c + 1) * 512]
                        )
                        nc.vector.tensor_add(
                            x2[:, qt, c * 512:(c + 1) * 512],
                            t1,
                            xq_sb[:, qt, c * 512:(c + 1) * 512],
                        )

            # ---- LN2 + MLP + residual ----
            with (
                tc.tile_pool(name="lnp2", bufs=2) as lnp2,
                tc.tile_pool(name="h2p", bufs=1) as h2p,
                tc.tile_pool(name="gp", bufs=1) as gp,
                tc.tile_pool(name="wfp", bufs=6) as wfp,
                tc.tile_pool(name="w2p", bufs=6) as w2p,
                tc.tile_pool(name="yp", bufs=2) as yp,
            ):
                h2T = h2p.tile([128, 8, 512], MMDT)
                G = gp.tile([128, 32, 512], MMDT)
                with (
                    tc.tile_pool(name="psT2", bufs=2, space="PSUM") as psT2,
                    tc.tile_pool(name="psF", bufs=4, space="PSUM") as psF,
                ):
                    _ln_transpose(nc, lnp2, psT2, ident, eps, x2, h2T, from_sbuf=True)

                    # MLP1: gelu(h2 @ w1 + b1), transposed output [dff, q]
                    for ft in range(32):
                        w1c = wfp.tile([128, 8, 128], MMDT, tag="w1")
                        nc.sync.dma_start(
                            out=w1c,
                            in_=W1[:, ft * 128:(ft + 1) * 128].rearrange(
                                "(t p) n -> p t n", p=128
                            ),
                        )
                        psf = psF.tile([128, 512], F32, tag="psF")
                        for dt in range(8):
                            nc.tensor.matmul(
                                psf, w1c[:, dt, :], h2T[:, dt, :],
                                start=(dt == 0), stop=(dt == 7),
                            )
                        nc.scalar.activation(
                            G[:, ft, :], psf, AF.Gelu, bias=b1T[:, ft:ft + 1]
                        )

                # MLP2: y = G^T @ w2 + b2 + x2
                with tc.tile_pool(name="psY", bufs=4, space="PSUM") as psY:
                  for c in range(2):
                    py = [psY.tile([128, 512], F32, tag="psY", name=f"py{c}_{i}") for i in range(4)]
                    for ft in range(32):
                        w2t = w2p.tile([128, 512], MMDT, tag="w2")
                        nc.sync.dma_start(
                            out=w2t,
                            in_=W2[ft * 128:(ft + 1) * 128, c * 512:(c + 1) * 512],
                        )
                        for qt in range(4):
                            nc.tensor.matmul(
                                py[qt], G[:, ft, qt * 128:(qt + 1) * 128], w2t,
                                start=(ft == 0), stop=(ft == 31),
                            )
                    for qt in range(4):
                        t1 = yp.tile([128, 512], F32, tag="yt1")
                        nc.vector.tensor_add(
                            t1, py[qt], b2_bc[:, c * 512:(c + 1) * 512]
                        )
                        yt = yp.tile([128, 512], F32, tag="yt2")
                        nc.vector.tensor_add(
                            yt, t1, x2[:, qt, c * 512:(c + 1) * 512]
                        )
                        nc.sync.dma_start(
                            out=Y[qt * 128:(qt + 1) * 128, c * 512:(c + 1) * 512],
                            in_=yt,
                        )

    nc.compile()
    return nc


_NC = None


def _get_nc():
    global _NC
    if _NC is None:
        _NC = _build()
    return _NC


def kernel(x, ln1_g, ln1_b, wq, bq, wk, bk, wv, bv, wo, bo, w1, b1, w2, b2, ln2_g, ln2_b):
    f32 = lambda a: np.ascontiguousarray(np.asarray(a, dtype=np.float32))
    x = f32(x)
    ln1_g, ln1_b = f32(ln1_g), f32(ln1_b)
    ln2_g, ln2_b = f32(ln2_g), f32(ln2_b)
    wq, wk, wv, wo = f32(wq), f32(wk), f32(wv), f32(wo)
    w1, w2 = f32(w1), f32(w2)
    bq, bk, bv, bo, b1, b2 = f32(bq), f32(bk), f32(bv), f32(bo), f32(b1), f32(b2)

    # Fold LayerNorm affine params into the following projections (exact).
    wq_e = f32(ln1_g[:, None] * wq)
    wk_e = f32(ln1_g[:, None] * wk)
    wv_e = f32(ln1_g[:, None] * wv)
    bq_e = f32(bq + ln1_b @ wq)
    bk_e = f32(bk + ln1_b @ wk)
    bv_e = f32(bv + ln1_b @ wv)
    w1_e = f32(ln2_g[:, None] * w1)
    b1_e = f32(b1 + ln2_b @ w1)

    common = {
        "wq": wq_e, "wk": wk_e, "wv": wv_e, "wo": wo,
        "w1": w1_e, "w2": w2,
        "bq": bq_e, "bk": bk_e, "bv": bv_e, "bo": bo,
        "b1": b1_e, "b2": b2,
    }
    in_maps = []
    for c in range(NCORES):
        b = c // 4
        qoff = (c % 4) * QT
        m = dict(common)
        m["xb"] = np.ascontiguousarray(x[b])
        m["xq"] = np.ascontiguousarray(x[b, qoff:qoff + QT])
        in_maps.append(m)

    nc = _get_nc()
    res = run_bass_kernel_spmd(nc, in_maps, core_ids=list(range(NCORES)))

    y = np.empty((B, S, D), dtype=np.float32)
    for c in range(NCORES):
        b = c // 4
        qoff = (c % 4) * QT
        y[b, qoff:qoff + QT] = res.results[c]["y"]
    return y
